# revision 3
# baseline (speedup 1.0000x reference)
"""Grouped GEMM (MoE routing) kernel for 8 Trainium2 NeuronCores.

out[off_g : off_g + size_g] = A[off_g : off_g + size_g] @ B[g]   for g in 0..63
A: [524288, 256] f32, B: [64, 256, 256] f32, groups are contiguous row ranges.

Strategy (hardcoded, from the sharding hint "expert-parallel"):
  - Every core runs an IDENTICAL static schedule of T rows; the per-core
    group assignment lives in the DATA (host-packed AT / BW tensors).
  - Packing: constrained LPT assigns 8 groups per core (balanced totals),
    then a local-search swap pass minimizes T.  Slot structure = 8 "bulk"
    slots (budget = min of the i-th largest group over cores) + ~7
    "cleanup" slots (sorted leftover tails, budget = max over cores).
    T ~ 66240 rows (1.1% padding) vs 69632 for one-group-per-slot octiles.
  - bf16 operands + bf16 output (accumulation stays f32 in PSUM).
  - Host packs each core's chunks back to back, pre-transposed to
    AT [256, T] bf16; device computes the TRANSPOSED output OUTT [256, T].
  - Uniform ~4K-row blocks tapered at both ends.  Per segment and output
    half h (128 of 256 N cols): stream <=512-row matmul spans (one PSUM
    bank each; h=0 rotates banks 0-3, h=1 banks 4-7), then cast-copy
    PSUM->SBUF on vector (h=0) / scalar (h=1) so both drain concurrently.
  - DMA queue topology is selectable (BASS_GG_TOPO); the last TAILROT
    blocks rotate stores over all 3 queues (loads are done by then) to
    speed the drain.
"""

import os
import numpy as np

NCORES = 8
K = 256
N = 256
SPAN = 512       # rows per PSUM bank (512 f32 = 2KB = one bank)

LAST_EXEC_NS = None  # set when BASS_GG_TRACE=1
LAST_EXEC_LIST = []

_prog_cache = {}
_sched_cache = {}


def _knobs():
    e = os.environ.get
    return dict(
        wrows=int(e("BASS_GG_W", "4096")),
        groupn=int(e("BASS_GG_GROUP", "4")),
        abufs=int(e("BASS_GG_ABUFS", "6")),
        obufs=int(e("BASS_GG_OBUFS", "6")),
        lookahead=int(e("BASS_GG_LOOKAHEAD", "5")),
        tailrot=int(e("BASS_GG_TAILROT", "6")),
        lead=tuple(int(x) for x in e("BASS_GG_LEAD", "1024,1024,2048").split(",") if x),
        tail=tuple(int(x) for x in e("BASS_GG_TAIL", "2048,1024,512,512").split(",") if x),
        cleanat=float(e("BASS_GG_CLEANAT", "0.55")),
        topo=e("BASS_GG_TOPO", "mixed"),
    )


def _schedule(sizes):
    """Pack groups into an identical-per-core slot schedule.

    Returns (r_list_raw, plan) where r_list_raw is the UNORDERED slot
    budget list [(budget, kind)] and plan[c][slot] = (group_id,
    group_row_start, nrows).  Groups padded to 64-row multiples.
    """
    import random

    key = tuple(int(x) for x in sizes)
    if key in _sched_cache:
        return _sched_cache[key]

    sizes = np.asarray(sizes, dtype=np.int64)
    g = sizes.shape[0]
    pad_groups = (-g) % NCORES
    if pad_groups:
        sizes = np.concatenate([sizes, np.zeros(pad_groups, np.int64)])
        g += pad_groups
    s64 = ((sizes + 63) // 64 * 64).astype(np.int64)
    per = g // NCORES

    order = np.argsort(-s64, kind="stable")
    tot = np.zeros(NCORES, np.int64)
    cnt = np.zeros(NCORES, np.int64)
    assign = [[] for _ in range(NCORES)]
    for gid in order:
        elig = [c for c in range(NCORES) if cnt[c] < per]
        c = min(elig, key=lambda c: tot[c])
        tot[c] += s64[gid]
        cnt[c] += 1
        assign[c].append(int(gid))

    def stats(asg):
        S = np.array(
            [sorted((s64[x] for x in asg[c]), reverse=True) for c in range(NCORES)],
            dtype=np.int64,
        )
        q = S.min(axis=0)
        L = S - q[None, :]
        p = (-np.sort(-L, axis=1)).max(axis=0)
        return int(q.sum() + p[p > 0].sum()), q, p

    best, _, _ = stats(assign)
    rnd = random.Random(7)
    cur = [list(a) for a in assign]
    for _ in range(12000):
        c1, c2 = rnd.sample(range(NCORES), 2)
        i1, i2 = rnd.randrange(per), rnd.randrange(per)
        cur[c1][i1], cur[c2][i2] = cur[c2][i2], cur[c1][i1]
        t, _, _ = stats(cur)
        if t <= best:
            best = t
        else:
            cur[c1][i1], cur[c2][i2] = cur[c2][i2], cur[c1][i1]

    T, q, p = stats(cur)
    nb = len(q)
    Gc = [sorted(cur[c], key=lambda x: -s64[x]) for c in range(NCORES)]
    keepp = [int(x) for x in p if x > 0]
    ncl = len(keepp)
    bulk = []
    for i in range(nb):
        entries = [(Gc[c][i], 0, int(q[i])) for c in range(NCORES)]
        bulk.append((int(q[i]), entries))
    lsorted = []
    for c in range(NCORES):
        lv = [(int(s64[Gc[c][i]] - q[i]), i) for i in range(nb)]
        lv.sort(key=lambda x: -x[0])
        lsorted.append(lv)
    clean = []
    for j in range(ncl):
        entries = []
        for c in range(NCORES):
            l, i = lsorted[c][j]
            entries.append((Gc[c][i], int(q[i]), l))
        clean.append((keepp[j], entries))

    _sched_cache[key] = (bulk, clean)
    return bulk, clean


def _order_slots(bulk, clean, cleanat):
    T = sum(b[0] for b in bulk) + sum(c[0] for c in clean)
    slots = []
    cum = 0
    inserted = False
    for bslot in bulk:
        if not inserted and cum >= cleanat * T:
            slots.extend(clean)
            inserted = True
        slots.append(bslot)
        cum += bslot[0]
    if not inserted:
        slots.extend(clean)
    r_list = [s[0] for s in slots]
    plan = [[s[1][c] for s in slots] for c in range(NCORES)]
    return r_list, plan


def _make_blocks(r_list, kn):
    T = int(sum(r_list))
    lead = list(kn["lead"])
    tail = list(kn["tail"])
    wrows = kn["wrows"]
    mid = T - sum(lead) - sum(tail)
    parts = max(1, (mid + wrows - 1) // wrows)
    base = (mid // parts + 63) // 64 * 64
    sizes = []
    rem = mid
    while rem > 0:
        w = min(base, rem)
        sizes.append(w)
        rem -= w
    blocks = []
    t0 = 0
    for w in lead + sizes + tail:
        blocks.append((t0, w))
        t0 += w
    assert t0 == T
    return blocks, T


def _build_program(r_list, kn):
    import concourse.tile as tile
    from concourse import bacc, mybir

    BF16 = mybir.dt.bfloat16
    F32 = mybir.dt.float32
    R = len(r_list)
    WROWS = kn["wrows"]
    GROUPN = kn["groupn"]
    TAILROT = kn["tailrot"]
    LOOKAHEAD = kn["lookahead"]

    blocks, T = _make_blocks(r_list, kn)
    slot_start = [0]
    for r in r_list:
        slot_start.append(slot_start[-1] + int(r))

    def slot_at(row):
        for i in range(R):
            if row < slot_start[i + 1]:
                return i
        return R - 1

    nc = bacc.Bacc(
        "TRN2",
        target_bir_lowering=False,
        debug=False,
        enable_asserts=False,
        num_devices=NCORES,
    )
    AT = nc.dram_tensor("AT", [K, T], BF16, kind="ExternalInput").ap()
    BW = nc.dram_tensor("BW", [128, R, 2, 2, 128], BF16, kind="ExternalInput").ap()
    OUTT = nc.dram_tensor("OUTT", [N, T], BF16, kind="ExternalOutput").ap()

    with tile.TileContext(nc) as tc:
        with tc.tile_pool(name="bpool", bufs=1) as bpool, \
             tc.tile_pool(name="apool", bufs=kn["abufs"]) as apool, \
             tc.tile_pool(name="opool", bufs=kn["obufs"]) as opool, \
             tc.tile_pool(name="psum", bufs=8, space="PSUM") as pspool:
            b_sb = bpool.tile([128, R, 2, 2, 128], BF16)

            abufs = {}
            ENG = [nc.sync, nc.scalar, nc.gpsimd]
            topo = kn["topo"]
            if topo == "split":
                qa0, qa1 = nc.gpsimd, nc.gpsimd
                qh0, qh1 = nc.sync, nc.sync
                wq = [nc.scalar, nc.scalar, nc.scalar]
            elif topo == "split2":
                qa0, qa1 = nc.gpsimd, nc.scalar
                qh0, qh1 = nc.sync, nc.sync
                wq = [nc.sync, nc.scalar, nc.gpsimd]
            else:  # mixed (baseline-like)
                qa0, qa1 = nc.sync, nc.scalar
                qh0, qh1 = nc.gpsimd, nc.sync
                wq = [nc.gpsimd, nc.scalar, nc.sync]

            def emit_loads(bi):
                t0, w = blocks[bi]
                a0 = apool.tile([128, WROWS], BF16, tag="a0")
                a1 = apool.tile([128, WROWS], BF16, tag="a1")
                qa0.dma_start(out=a0[:, :w], in_=AT[0:128, t0 : t0 + w])
                qa1.dma_start(out=a1[:, :w], in_=AT[128:256, t0 : t0 + w])
                abufs[bi] = (a0, a1)

            nblk = len(blocks)

            def emit_compute(bi):
                t0, w = blocks[bi]
                a0, a1 = abufs.pop(bi)
                ob = opool.tile([128, 2, WROWS], BF16, tag="ob")

                segs = []
                off = 0
                while off < w:
                    s = slot_at(t0 + off)
                    end = min(w, slot_start[s + 1] - t0)
                    segs.append((off, end, s))
                    off = end

                for (so, se, s) in segs:
                    spans = []
                    off = so
                    while off < se:
                        spans.append((off, min(SPAN, se - off)))
                        off += spans[-1][1]
                    for c0 in range(0, len(spans), GROUPN):
                        chunk = spans[c0 : c0 + GROUPN]
                        for h in range(2):
                            pss = [
                                pspool.tile([128, SPAN], F32, name="ps")
                                for _ in chunk
                            ]
                            for j, aj in ((0, a0), (1, a1)):
                                for (off, ln), ps in zip(chunk, pss):
                                    nc.tensor.matmul(
                                        ps[:, :ln],
                                        lhsT=b_sb[:, s, j, h, :],
                                        rhs=aj[:, off : off + ln],
                                        start=(j == 0),
                                        stop=(j == 1),
                                    )
                            eng = (
                                nc.vector.tensor_copy if h == 0 else nc.scalar.copy
                            )
                            for (off, ln), ps in zip(chunk, pss):
                                eng(out=ob[:, h, off : off + ln], in_=ps[:, :ln])
                if TAILROT and bi >= nblk - TAILROT:
                    qs0 = ENG[(2 * bi) % 3]
                    qs1 = ENG[(2 * bi + 1) % 3]
                else:
                    qs0, qs1 = qh0, qh1
                for h, qq in ((0, qs0), (1, qs1)):
                    qq.dma_start(
                        out=OUTT[h * 128 : (h + 1) * 128, t0 : t0 + w],
                        in_=ob[:, h, :w],
                    )

            # head: first two blocks' loads before the weights; weights in
            # schedule order so early slots arrive first
            emit_loads(0)
            emit_loads(1)
            cuts = [0, min(2, R), min(6, R), min(11, R), R]
            wqs = [wq[0], wq[0], wq[1], wq[2]]
            for (lo, hi), qq in zip(zip(cuts, cuts[1:]), wqs):
                if hi > lo:
                    qq.dma_start(out=b_sb[:, lo:hi], in_=BW[:, lo:hi])

            for bi in range(2, nblk + LOOKAHEAD):
                if bi < nblk:
                    emit_loads(bi)
                if bi >= LOOKAHEAD:
                    emit_compute(bi - LOOKAHEAD)
    nc.compile()
    return nc


def _get_program(r_key, kn):
    key = (r_key, tuple(sorted(kn.items())))
    if key not in _prog_cache:
        _prog_cache[key] = _build_program(list(r_key), kn)
    return _prog_cache[key]


def kernel(A, B, batch_sizes, batch_offsets, batch_padded_offsets):
    global LAST_EXEC_NS
    import ml_dtypes
    from concourse.bass_utils import run_bass_kernel_spmd

    try:
        import antenv.axon_hooks  # noqa: F401
    except ImportError:
        import sys
        import types

        _m = types.ModuleType("antenv.axon_hooks")
        _m.get_axon_ntff_profile_hook = lambda: None
        sys.modules.setdefault("antenv.axon_hooks", _m)

    kn = _knobs()
    bf16 = ml_dtypes.bfloat16
    A = np.asarray(A, dtype=np.float32)
    B = np.asarray(B, dtype=np.float32)
    sizes = np.asarray(batch_sizes, dtype=np.int64)
    offsets = np.asarray(batch_offsets, dtype=np.int64)

    M = A.shape[0]
    G = B.shape[0]
    bulk, clean = _schedule(sizes)
    r_list, plan = _order_slots(bulk, clean, kn["cleanat"])
    starts = np.concatenate([[0], np.cumsum(r_list)[:-1]]).astype(np.int64)
    T = int(sum(r_list))
    R = len(r_list)

    nc = _get_program(tuple(int(x) for x in r_list), kn)

    ATfull = np.ascontiguousarray(A.astype(bf16).T)  # [K, M]
    Bbf = B.astype(bf16)  # [G, K, N]

    in_maps = []
    for c in range(NCORES):
        at = np.zeros((K, T), dtype=bf16)
        bw = np.zeros((128, R, 2, 2, 128), dtype=bf16)
        for i in range(R):
            gid, gr0, nrows = plan[c][i]
            dst = int(starts[i])
            if gid < G:
                off, sz = int(offsets[gid]), int(sizes[gid])
                lo = min(gr0, sz)
                hi = min(gr0 + nrows, sz)
                if hi > lo:
                    at[:, dst + (lo - gr0) : dst + (hi - gr0)] = ATfull[
                        :, off + lo : off + hi
                    ]
                bw[:, i] = Bbf[gid].reshape(2, 128, 2, 128).transpose(1, 0, 2, 3)
        in_maps.append({"AT": at, "BW": bw})

    trace = bool(int(os.environ.get("BASS_GG_TRACE", "0")))
    repeats = int(os.environ.get("BASS_GG_REPEAT", "1"))
    times = []
    for _ in range(max(1, repeats)):
        res = run_bass_kernel_spmd(
            nc, in_maps, core_ids=list(range(NCORES)), trace=trace
        )
        times.append(res.exec_time_ns)
    global LAST_EXEC_LIST
    LAST_EXEC_LIST = times
    LAST_EXEC_NS = min((t for t in times if t is not None), default=None)

    outT = np.zeros((N, M), dtype=np.float32)
    for c in range(NCORES):
        oc = res.results[c]["OUTT"]
        for i in range(R):
            gid, gr0, nrows = plan[c][i]
            src = int(starts[i])
            if gid >= G:
                continue
            off, sz = int(offsets[gid]), int(sizes[gid])
            lo = min(gr0, sz)
            hi = min(gr0 + nrows, sz)
            if hi > lo:
                outT[:, off + lo : off + hi] = oc[
                    :, src + (lo - gr0) : src + (hi - gr0)
                ]
    return outT.T


# revision 7
# speedup vs baseline: 1.0113x; 1.0113x over previous
"""Grouped GEMM (MoE routing) kernel for 8 Trainium2 NeuronCores.

out[off_g : off_g + size_g] = A[off_g : off_g + size_g] @ B[g]   for g in 0..63
A: [524288, 256] f32, B: [64, 256, 256] f32, groups are contiguous row ranges.

Strategy (hardcoded, from the sharding hint "expert-parallel"):
  - Every core runs an IDENTICAL static schedule of T rows; the per-core
    group assignment lives in the DATA (host-packed AT / BW tensors).
  - Packing: constrained LPT assigns 8 groups per core (balanced totals),
    then a local-search swap pass minimizes T.  Slot structure = 8 "bulk"
    slots (budget = min of the i-th largest group over cores) + ~7
    "cleanup" slots (sorted leftover tails, budget = max over cores).
    T ~ 66240 rows (1.1% padding) vs 69632 for one-group-per-slot octiles.
  - bf16 operands + bf16 output (accumulation stays f32 in PSUM).
  - Host packs each core's chunks back to back, pre-transposed to
    AT [256, T] bf16; device computes the TRANSPOSED output OUTT [256, T].
  - Uniform ~4K-row blocks tapered at both ends.  Per segment and output
    half h (128 of 256 N cols): stream <=512-row matmul spans (one PSUM
    bank each; h=0 rotates banks 0-3, h=1 banks 4-7), then cast-copy
    PSUM->SBUF on vector (h=0) / scalar (h=1) so both drain concurrently.
    GROUPN=2 spans per PSUM group: shorter PE bursts interleave better
    with the copy engines and, empirically, largely avoid the ~50%-util
    activity-throttle windows that add ~25us to GROUPN=4 runs.
  - DMA queue topology is selectable (BASS_GG_TOPO); the last TAILROT
    blocks rotate stores over all 3 queues (loads are done by then) to
    speed the drain.
"""

import os
import numpy as np

NCORES = 8
K = 256
N = 256
SPAN = 512       # rows per PSUM bank (512 f32 = 2KB = one bank)

LAST_EXEC_NS = None  # set when BASS_GG_TRACE=1
LAST_EXEC_LIST = []

_prog_cache = {}
_sched_cache = {}


def _knobs():
    e = os.environ.get
    return dict(
        wrows=int(e("BASS_GG_W", "4096")),
        groupn=int(e("BASS_GG_GROUP", "2")),
        abufs=int(e("BASS_GG_ABUFS", "6")),
        obufs=int(e("BASS_GG_OBUFS", "6")),
        lookahead=int(e("BASS_GG_LOOKAHEAD", "5")),
        tailrot=int(e("BASS_GG_TAILROT", "6")),
        lead=tuple(int(x) for x in e("BASS_GG_LEAD", "1024,1024,2048").split(",") if x),
        tail=tuple(int(x) for x in e("BASS_GG_TAIL", "2048,1024,512,512").split(",") if x),
        cleanat=float(e("BASS_GG_CLEANAT", "0.55")),
        topo=e("BASS_GG_TOPO", "mixed"),
        ssplit=int(e("BASS_GG_SSPLIT", "1")),
    )


def _schedule(sizes):
    """Pack groups into an identical-per-core slot schedule.

    Returns (r_list_raw, plan) where r_list_raw is the UNORDERED slot
    budget list [(budget, kind)] and plan[c][slot] = (group_id,
    group_row_start, nrows).  Groups padded to 64-row multiples.
    """
    import random

    key = tuple(int(x) for x in sizes)
    if key in _sched_cache:
        return _sched_cache[key]

    sizes = np.asarray(sizes, dtype=np.int64)
    g = sizes.shape[0]
    pad_groups = (-g) % NCORES
    if pad_groups:
        sizes = np.concatenate([sizes, np.zeros(pad_groups, np.int64)])
        g += pad_groups
    s64 = ((sizes + 63) // 64 * 64).astype(np.int64)
    per = g // NCORES

    order = np.argsort(-s64, kind="stable")
    tot = np.zeros(NCORES, np.int64)
    cnt = np.zeros(NCORES, np.int64)
    assign = [[] for _ in range(NCORES)]
    for gid in order:
        elig = [c for c in range(NCORES) if cnt[c] < per]
        c = min(elig, key=lambda c: tot[c])
        tot[c] += s64[gid]
        cnt[c] += 1
        assign[c].append(int(gid))

    def stats(asg):
        S = np.array(
            [sorted((s64[x] for x in asg[c]), reverse=True) for c in range(NCORES)],
            dtype=np.int64,
        )
        q = S.min(axis=0)
        L = S - q[None, :]
        p = (-np.sort(-L, axis=1)).max(axis=0)
        return int(q.sum() + p[p > 0].sum()), q, p

    best, _, _ = stats(assign)
    rnd = random.Random(7)
    cur = [list(a) for a in assign]
    for _ in range(12000):
        c1, c2 = rnd.sample(range(NCORES), 2)
        i1, i2 = rnd.randrange(per), rnd.randrange(per)
        cur[c1][i1], cur[c2][i2] = cur[c2][i2], cur[c1][i1]
        t, _, _ = stats(cur)
        if t <= best:
            best = t
        else:
            cur[c1][i1], cur[c2][i2] = cur[c2][i2], cur[c1][i1]

    T, q, p = stats(cur)
    nb = len(q)
    Gc = [sorted(cur[c], key=lambda x: -s64[x]) for c in range(NCORES)]
    keepp = [int(x) for x in p if x > 0]
    ncl = len(keepp)
    bulk = []
    for i in range(nb):
        entries = [(Gc[c][i], 0, int(q[i])) for c in range(NCORES)]
        bulk.append((int(q[i]), entries))
    lsorted = []
    for c in range(NCORES):
        lv = [(int(s64[Gc[c][i]] - q[i]), i) for i in range(nb)]
        lv.sort(key=lambda x: -x[0])
        lsorted.append(lv)
    clean = []
    for j in range(ncl):
        entries = []
        for c in range(NCORES):
            l, i = lsorted[c][j]
            entries.append((Gc[c][i], int(q[i]), l))
        clean.append((keepp[j], entries))

    _sched_cache[key] = (bulk, clean)
    return bulk, clean


def _order_slots(bulk, clean, cleanat):
    T = sum(b[0] for b in bulk) + sum(c[0] for c in clean)
    slots = []
    cum = 0
    inserted = False
    for bslot in bulk:
        if not inserted and cum >= cleanat * T:
            slots.extend(clean)
            inserted = True
        slots.append(bslot)
        cum += bslot[0]
    if not inserted:
        slots.extend(clean)
    r_list = [s[0] for s in slots]
    plan = [[s[1][c] for s in slots] for c in range(NCORES)]
    return r_list, plan


def _make_blocks(r_list, kn):
    T = int(sum(r_list))
    lead = list(kn["lead"])
    tail = list(kn["tail"])
    wrows = kn["wrows"]
    mid = T - sum(lead) - sum(tail)
    parts = max(1, (mid + wrows - 1) // wrows)
    base = (mid // parts + 63) // 64 * 64
    sizes = []
    rem = mid
    while rem > 0:
        w = min(base, rem)
        sizes.append(w)
        rem -= w
    blocks = []
    t0 = 0
    for w in lead + sizes + tail:
        blocks.append((t0, w))
        t0 += w
    assert t0 == T
    return blocks, T


def _build_program(r_list, kn):
    import concourse.tile as tile
    from concourse import bacc, mybir

    BF16 = mybir.dt.bfloat16
    F32 = mybir.dt.float32
    R = len(r_list)
    WROWS = kn["wrows"]
    GROUPN = kn["groupn"]
    TAILROT = kn["tailrot"]
    LOOKAHEAD = kn["lookahead"]

    blocks, T = _make_blocks(r_list, kn)
    slot_start = [0]
    for r in r_list:
        slot_start.append(slot_start[-1] + int(r))

    def slot_at(row):
        for i in range(R):
            if row < slot_start[i + 1]:
                return i
        return R - 1

    nc = bacc.Bacc(
        "TRN2",
        target_bir_lowering=False,
        debug=False,
        enable_asserts=False,
        num_devices=NCORES,
    )
    AT = nc.dram_tensor("AT", [K, T], BF16, kind="ExternalInput").ap()
    BW = nc.dram_tensor("BW", [128, R, 2, 2, 128], BF16, kind="ExternalInput").ap()
    OUTT = nc.dram_tensor("OUTT", [N, T], BF16, kind="ExternalOutput").ap()

    with tile.TileContext(nc) as tc:
        with tc.tile_pool(name="bpool", bufs=1) as bpool, \
             tc.tile_pool(name="apool", bufs=kn["abufs"]) as apool, \
             tc.tile_pool(name="opool", bufs=kn["obufs"]) as opool, \
             tc.tile_pool(name="psum", bufs=8, space="PSUM") as pspool:
            b_sb = bpool.tile([128, R, 2, 2, 128], BF16)

            abufs = {}
            ENG = [nc.sync, nc.scalar, nc.gpsimd]
            topo = kn["topo"]
            if topo == "split":
                qa0, qa1 = nc.gpsimd, nc.gpsimd
                qh0, qh1 = nc.sync, nc.sync
                wq = [nc.scalar, nc.scalar, nc.scalar]
            elif topo == "split2":
                qa0, qa1 = nc.gpsimd, nc.scalar
                qh0, qh1 = nc.sync, nc.sync
                wq = [nc.sync, nc.scalar, nc.gpsimd]
            else:  # mixed (baseline-like)
                qa0, qa1 = nc.sync, nc.scalar
                qh0, qh1 = nc.gpsimd, nc.sync
                wq = [nc.gpsimd, nc.scalar, nc.sync]

            def emit_loads(bi):
                t0, w = blocks[bi]
                a0 = apool.tile([128, WROWS], BF16, tag="a0")
                a1 = apool.tile([128, WROWS], BF16, tag="a1")
                qa0.dma_start(out=a0[:, :w], in_=AT[0:128, t0 : t0 + w])
                qa1.dma_start(out=a1[:, :w], in_=AT[128:256, t0 : t0 + w])
                abufs[bi] = (a0, a1)

            nblk = len(blocks)

            def emit_compute(bi):
                t0, w = blocks[bi]
                a0, a1 = abufs.pop(bi)
                ob = opool.tile([128, 2, WROWS], BF16, tag="ob")

                segs = []
                off = 0
                while off < w:
                    s = slot_at(t0 + off)
                    end = min(w, slot_start[s + 1] - t0)
                    segs.append((off, end, s))
                    off = end

                for (so, se, s) in segs:
                    spans = []
                    off = so
                    while off < se:
                        spans.append((off, min(SPAN, se - off)))
                        off += spans[-1][1]
                    for c0 in range(0, len(spans), GROUPN):
                        chunk = spans[c0 : c0 + GROUPN]
                        for h in range(2):
                            pss = [
                                pspool.tile([128, SPAN], F32, name="ps")
                                for _ in chunk
                            ]
                            for j, aj in ((0, a0), (1, a1)):
                                for (off, ln), ps in zip(chunk, pss):
                                    nc.tensor.matmul(
                                        ps[:, :ln],
                                        lhsT=b_sb[:, s, j, h, :],
                                        rhs=aj[:, off : off + ln],
                                        start=(j == 0),
                                        stop=(j == 1),
                                    )
                            eng = (
                                nc.vector.tensor_copy if h == 0 else nc.scalar.copy
                            )
                            for (off, ln), ps in zip(chunk, pss):
                                eng(out=ob[:, h, off : off + ln], in_=ps[:, :ln])
                intail = TAILROT and bi >= nblk - TAILROT
                if intail:
                    qs0 = ENG[(2 * bi) % 3]
                    qs1 = ENG[(2 * bi + 1) % 3]
                else:
                    qs0, qs1 = qh0, qh1
                nsp = kn["ssplit"] if w >= 2048 else 1
                cuts = [w * i // nsp // 64 * 64 for i in range(nsp)] + [w]
                for ci in range(nsp):
                    lo, hi = cuts[ci], cuts[ci + 1]
                    for h, qq in ((0, qs0), (1, qs1)):
                        if intail:
                            qq = ENG[(2 * bi + 2 * ci + h) % 3]
                        qq.dma_start(
                            out=OUTT[h * 128 : (h + 1) * 128, t0 + lo : t0 + hi],
                            in_=ob[:, h, lo:hi],
                        )

            # head: first two blocks' loads before the weights; weights in
            # schedule order so early slots arrive first
            emit_loads(0)
            emit_loads(1)
            cuts = [0, min(2, R), min(6, R), min(11, R), R]
            wqs = [wq[0], wq[0], wq[1], wq[2]]
            for (lo, hi), qq in zip(zip(cuts, cuts[1:]), wqs):
                if hi > lo:
                    qq.dma_start(out=b_sb[:, lo:hi], in_=BW[:, lo:hi])

            for bi in range(2, nblk + LOOKAHEAD):
                if bi < nblk:
                    emit_loads(bi)
                if bi >= LOOKAHEAD:
                    emit_compute(bi - LOOKAHEAD)
    nc.compile()
    return nc


def _get_program(r_key, kn):
    key = (r_key, tuple(sorted(kn.items())))
    if key not in _prog_cache:
        _prog_cache[key] = _build_program(list(r_key), kn)
    return _prog_cache[key]


def kernel(A, B, batch_sizes, batch_offsets, batch_padded_offsets):
    global LAST_EXEC_NS
    import ml_dtypes
    from concourse.bass_utils import run_bass_kernel_spmd

    try:
        import antenv.axon_hooks  # noqa: F401
    except ImportError:
        import sys
        import types

        _m = types.ModuleType("antenv.axon_hooks")
        _m.get_axon_ntff_profile_hook = lambda: None
        sys.modules.setdefault("antenv.axon_hooks", _m)

    kn = _knobs()
    bf16 = ml_dtypes.bfloat16
    A = np.asarray(A, dtype=np.float32)
    B = np.asarray(B, dtype=np.float32)
    sizes = np.asarray(batch_sizes, dtype=np.int64)
    offsets = np.asarray(batch_offsets, dtype=np.int64)

    M = A.shape[0]
    G = B.shape[0]
    bulk, clean = _schedule(sizes)
    r_list, plan = _order_slots(bulk, clean, kn["cleanat"])
    starts = np.concatenate([[0], np.cumsum(r_list)[:-1]]).astype(np.int64)
    T = int(sum(r_list))
    R = len(r_list)

    nc = _get_program(tuple(int(x) for x in r_list), kn)

    ATfull = np.ascontiguousarray(A.astype(bf16).T)  # [K, M]
    Bbf = B.astype(bf16)  # [G, K, N]

    in_maps = []
    for c in range(NCORES):
        at = np.zeros((K, T), dtype=bf16)
        bw = np.zeros((128, R, 2, 2, 128), dtype=bf16)
        for i in range(R):
            gid, gr0, nrows = plan[c][i]
            dst = int(starts[i])
            if gid < G:
                off, sz = int(offsets[gid]), int(sizes[gid])
                lo = min(gr0, sz)
                hi = min(gr0 + nrows, sz)
                if hi > lo:
                    at[:, dst + (lo - gr0) : dst + (hi - gr0)] = ATfull[
                        :, off + lo : off + hi
                    ]
                bw[:, i] = Bbf[gid].reshape(2, 128, 2, 128).transpose(1, 0, 2, 3)
        in_maps.append({"AT": at, "BW": bw})

    trace = bool(int(os.environ.get("BASS_GG_TRACE", "0")))
    repeats = int(os.environ.get("BASS_GG_REPEAT", "1"))
    times = []
    for _ in range(max(1, repeats)):
        res = run_bass_kernel_spmd(
            nc, in_maps, core_ids=list(range(NCORES)), trace=trace
        )
        times.append(res.exec_time_ns)
    global LAST_EXEC_LIST
    LAST_EXEC_LIST = times
    LAST_EXEC_NS = min((t for t in times if t is not None), default=None)

    outT = np.zeros((N, M), dtype=np.float32)
    for c in range(NCORES):
        oc = res.results[c]["OUTT"]
        for i in range(R):
            gid, gr0, nrows = plan[c][i]
            src = int(starts[i])
            if gid >= G:
                continue
            off, sz = int(offsets[gid]), int(sizes[gid])
            lo = min(gr0, sz)
            hi = min(gr0 + nrows, sz)
            if hi > lo:
                outT[:, off + lo : off + hi] = oc[
                    :, src + (lo - gr0) : src + (hi - gr0)
                ]
    return outT.T


# revision 8
# speedup vs baseline: 1.1234x; 1.1108x over previous
"""Grouped GEMM (MoE routing) kernel for 8 Trainium2 NeuronCores.

out[off_g : off_g + size_g] = A[off_g : off_g + size_g] @ B[g]   for g in 0..63
A: [524288, 256] f32, B: [64, 256, 256] f32, groups are contiguous row ranges.

Strategy (hardcoded, from the sharding hint "expert-parallel"):
  - Every core runs an IDENTICAL static schedule of T rows; the per-core
    group assignment lives in the DATA (host-packed AT / BW tensors).
  - Packing: constrained LPT assigns 8 groups per core (balanced totals),
    then a local-search swap pass minimizes T.  Slot structure = 8 "bulk"
    slots (budget = min of the i-th largest group over cores) + ~7
    "cleanup" slots (sorted leftover tails, budget = max over cores).
    T ~ 66240 rows (1.1% padding) vs 69632 for one-group-per-slot octiles.
  - bf16 operands + bf16 output (accumulation stays f32 in PSUM).
  - Host packs each core's chunks back to back, pre-transposed to
    AT [256, T] bf16; device computes the TRANSPOSED output OUTT [256, T].
  - Uniform ~4K-row blocks tapered at both ends.  Per segment and output
    half h (128 of 256 N cols): stream <=512-row matmul spans (one PSUM
    bank each; h=0 rotates banks 0-3, h=1 banks 4-7), then cast-copy
    PSUM->SBUF on vector (h=0) / scalar (h=1) so both drain concurrently.
    GROUPN=2 spans per PSUM group: shorter PE bursts interleave better
    with the copy engines and, empirically, largely avoid the ~50%-util
    activity-throttle windows that add ~25us to GROUPN=4 runs.
  - DMA queue topology is selectable (BASS_GG_TOPO); the last TAILROT
    blocks rotate stores over all 3 queues (loads are done by then) to
    speed the drain.
"""

import os
import numpy as np

NCORES = 8
K = 256
N = 256
SPAN = 512       # rows per PSUM bank (512 f32 = 2KB = one bank)

LAST_EXEC_NS = None  # set when BASS_GG_TRACE=1
LAST_EXEC_LIST = []

_prog_cache = {}
_sched_cache = {}


def _knobs():
    e = os.environ.get
    return dict(
        wrows=int(e("BASS_GG_W", "4096")),
        groupn=int(e("BASS_GG_GROUP", "2")),
        abufs=int(e("BASS_GG_ABUFS", "6")),
        obufs=int(e("BASS_GG_OBUFS", "6")),
        lookahead=int(e("BASS_GG_LOOKAHEAD", "5")),
        tailrot=int(e("BASS_GG_TAILROT", "6")),
        lead=tuple(int(x) for x in e("BASS_GG_LEAD", "1024,1024,2048").split(",") if x),
        tail=tuple(int(x) for x in e("BASS_GG_TAIL", "2048,1024,512,512").split(",") if x),
        cleanat=float(e("BASS_GG_CLEANAT", "0.55")),
        topo=e("BASS_GG_TOPO", "mixed"),
        ssplit=int(e("BASS_GG_SSPLIT", "1")),
    )


def _schedule(sizes):
    """Pack groups into an identical-per-core slot schedule.

    Returns (r_list_raw, plan) where r_list_raw is the UNORDERED slot
    budget list [(budget, kind)] and plan[c][slot] = (group_id,
    group_row_start, nrows).  Groups padded to 64-row multiples.
    """
    import random

    key = tuple(int(x) for x in sizes)
    if key in _sched_cache:
        return _sched_cache[key]

    sizes = np.asarray(sizes, dtype=np.int64)
    g = sizes.shape[0]
    pad_groups = (-g) % NCORES
    if pad_groups:
        sizes = np.concatenate([sizes, np.zeros(pad_groups, np.int64)])
        g += pad_groups
    s64 = ((sizes + 63) // 64 * 64).astype(np.int64)
    per = g // NCORES

    order = np.argsort(-s64, kind="stable")
    tot = np.zeros(NCORES, np.int64)
    cnt = np.zeros(NCORES, np.int64)
    assign = [[] for _ in range(NCORES)]
    for gid in order:
        elig = [c for c in range(NCORES) if cnt[c] < per]
        c = min(elig, key=lambda c: tot[c])
        tot[c] += s64[gid]
        cnt[c] += 1
        assign[c].append(int(gid))

    def stats(asg):
        S = np.array(
            [sorted((s64[x] for x in asg[c]), reverse=True) for c in range(NCORES)],
            dtype=np.int64,
        )
        q = S.min(axis=0)
        L = S - q[None, :]
        p = (-np.sort(-L, axis=1)).max(axis=0)
        return int(q.sum() + p[p > 0].sum()), q, p

    best, _, _ = stats(assign)
    rnd = random.Random(7)
    cur = [list(a) for a in assign]
    for _ in range(12000):
        c1, c2 = rnd.sample(range(NCORES), 2)
        i1, i2 = rnd.randrange(per), rnd.randrange(per)
        cur[c1][i1], cur[c2][i2] = cur[c2][i2], cur[c1][i1]
        t, _, _ = stats(cur)
        if t <= best:
            best = t
        else:
            cur[c1][i1], cur[c2][i2] = cur[c2][i2], cur[c1][i1]

    T, q, p = stats(cur)
    nb = len(q)
    Gc = [sorted(cur[c], key=lambda x: -s64[x]) for c in range(NCORES)]
    keepp = [int(x) for x in p if x > 0]
    ncl = len(keepp)
    bulk = []
    for i in range(nb):
        entries = [(Gc[c][i], 0, int(q[i])) for c in range(NCORES)]
        bulk.append((int(q[i]), entries))
    lsorted = []
    for c in range(NCORES):
        lv = [(int(s64[Gc[c][i]] - q[i]), i) for i in range(nb)]
        lv.sort(key=lambda x: -x[0])
        lsorted.append(lv)
    clean = []
    for j in range(ncl):
        entries = []
        for c in range(NCORES):
            l, i = lsorted[c][j]
            entries.append((Gc[c][i], int(q[i]), l))
        clean.append((keepp[j], entries))

    _sched_cache[key] = (bulk, clean)
    return bulk, clean


def _order_slots(bulk, clean, cleanat):
    T = sum(b[0] for b in bulk) + sum(c[0] for c in clean)
    slots = []
    cum = 0
    inserted = False
    for bslot in bulk:
        if not inserted and cum >= cleanat * T:
            slots.extend(clean)
            inserted = True
        slots.append(bslot)
        cum += bslot[0]
    if not inserted:
        slots.extend(clean)
    r_list = [s[0] for s in slots]
    plan = [[s[1][c] for s in slots] for c in range(NCORES)]
    return r_list, plan


def _make_blocks(r_list, kn):
    T = int(sum(r_list))
    lead = list(kn["lead"])
    tail = list(kn["tail"])
    wrows = kn["wrows"]
    mid = T - sum(lead) - sum(tail)
    parts = max(1, (mid + wrows - 1) // wrows)
    base = (mid // parts + 63) // 64 * 64
    sizes = []
    rem = mid
    while rem > 0:
        w = min(base, rem)
        sizes.append(w)
        rem -= w
    blocks = []
    t0 = 0
    for w in lead + sizes + tail:
        blocks.append((t0, w))
        t0 += w
    assert t0 == T
    return blocks, T


def _build_program(r_list, kn):
    import concourse.tile as tile
    from concourse import bacc, mybir

    BF16 = mybir.dt.bfloat16
    F32 = mybir.dt.float32
    R = len(r_list)
    WROWS = kn["wrows"]
    GROUPN = kn["groupn"]
    TAILROT = kn["tailrot"]
    LOOKAHEAD = kn["lookahead"]

    blocks, T = _make_blocks(r_list, kn)
    slot_start = [0]
    for r in r_list:
        slot_start.append(slot_start[-1] + int(r))

    def slot_at(row):
        for i in range(R):
            if row < slot_start[i + 1]:
                return i
        return R - 1

    nc = bacc.Bacc(
        "TRN2",
        target_bir_lowering=False,
        debug=False,
        enable_asserts=False,
        num_devices=NCORES,
    )
    AT = nc.dram_tensor("AT", [K, T], BF16, kind="ExternalInput").ap()
    BW = nc.dram_tensor("BW", [128, R, 2, 2, 128], BF16, kind="ExternalInput").ap()
    OUTT = nc.dram_tensor("OUTT", [N, T], BF16, kind="ExternalOutput").ap()

    with tile.TileContext(nc) as tc:
        with tc.tile_pool(name="bpool", bufs=1) as bpool, \
             tc.tile_pool(name="apool", bufs=kn["abufs"]) as apool, \
             tc.tile_pool(name="opool", bufs=kn["obufs"]) as opool, \
             tc.tile_pool(name="psum", bufs=8, space="PSUM") as pspool:
            b_sb = bpool.tile([128, R, 2, 2, 128], BF16)

            abufs = {}
            ENG = [nc.sync, nc.scalar, nc.gpsimd]
            topo = kn["topo"]
            if topo == "split":
                qa0, qa1 = nc.gpsimd, nc.gpsimd
                qh0, qh1 = nc.sync, nc.sync
                wq = [nc.scalar, nc.scalar, nc.scalar]
            elif topo == "split2":
                qa0, qa1 = nc.gpsimd, nc.scalar
                qh0, qh1 = nc.sync, nc.sync
                wq = [nc.sync, nc.scalar, nc.gpsimd]
            else:  # mixed (baseline-like)
                qa0, qa1 = nc.sync, nc.scalar
                qh0, qh1 = nc.gpsimd, nc.sync
                wq = [nc.gpsimd, nc.scalar, nc.sync]

            def emit_loads(bi):
                t0, w = blocks[bi]
                a0 = apool.tile([128, WROWS], BF16, tag="a0")
                a1 = apool.tile([128, WROWS], BF16, tag="a1")
                qa0.dma_start(out=a0[:, :w], in_=AT[0:128, t0 : t0 + w])
                qa1.dma_start(out=a1[:, :w], in_=AT[128:256, t0 : t0 + w])
                abufs[bi] = (a0, a1)

            nblk = len(blocks)

            def emit_compute(bi):
                t0, w = blocks[bi]
                a0, a1 = abufs.pop(bi)
                ob = opool.tile([128, 2, WROWS], BF16, tag="ob")

                segs = []
                off = 0
                while off < w:
                    s = slot_at(t0 + off)
                    end = min(w, slot_start[s + 1] - t0)
                    segs.append((off, end, s))
                    off = end

                for (so, se, s) in segs:
                    spans = []
                    off = so
                    while off < se:
                        spans.append((off, min(SPAN, se - off)))
                        off += spans[-1][1]
                    for c0 in range(0, len(spans), GROUPN):
                        chunk = spans[c0 : c0 + GROUPN]
                        for h in range(2):
                            pss = [
                                pspool.tile([128, SPAN], F32, name="ps")
                                for _ in chunk
                            ]
                            for j, aj in ((0, a0), (1, a1)):
                                for (off, ln), ps in zip(chunk, pss):
                                    nc.tensor.matmul(
                                        ps[:, :ln],
                                        lhsT=b_sb[:, s, j, h, :],
                                        rhs=aj[:, off : off + ln],
                                        start=(j == 0),
                                        stop=(j == 1),
                                    )
                            eng = (
                                nc.vector.tensor_copy if h == 0 else nc.scalar.copy
                            )
                            for (off, ln), ps in zip(chunk, pss):
                                eng(out=ob[:, h, off : off + ln], in_=ps[:, :ln])
                intail = TAILROT and bi >= nblk - TAILROT
                if intail:
                    # rotate from the end so the final block always lands on
                    # the two HWDGE rings (sync+scalar: fastest trigger and
                    # completion), the 2nd-to-last on gpsimd+sync, ...
                    k = nblk - 1 - bi
                    qs0 = ENG[(2 * k) % 3]
                    qs1 = ENG[(2 * k + 1) % 3]
                else:
                    qs0, qs1 = qh0, qh1
                nsp = kn["ssplit"] if w >= 2048 else 1
                cuts = [w * i // nsp // 64 * 64 for i in range(nsp)] + [w]
                for ci in range(nsp):
                    lo, hi = cuts[ci], cuts[ci + 1]
                    for h, qq in ((0, qs0), (1, qs1)):
                        if intail:
                            qq = ENG[(2 * bi + 2 * ci + h) % 3]
                        qq.dma_start(
                            out=OUTT[h * 128 : (h + 1) * 128, t0 + lo : t0 + hi],
                            in_=ob[:, h, lo:hi],
                        )

            # head: first two blocks' loads before the weights; weights in
            # schedule order so early slots arrive first
            emit_loads(0)
            emit_loads(1)
            cuts = [0, min(2, R), min(6, R), min(11, R), R]
            wqs = [wq[0], wq[0], wq[1], wq[2]]
            for (lo, hi), qq in zip(zip(cuts, cuts[1:]), wqs):
                if hi > lo:
                    qq.dma_start(out=b_sb[:, lo:hi], in_=BW[:, lo:hi])

            for bi in range(2, nblk + LOOKAHEAD):
                if bi < nblk:
                    emit_loads(bi)
                if bi >= LOOKAHEAD:
                    emit_compute(bi - LOOKAHEAD)
    nc.compile()
    return nc


def _get_program(r_key, kn):
    key = (r_key, tuple(sorted(kn.items())))
    if key not in _prog_cache:
        _prog_cache[key] = _build_program(list(r_key), kn)
    return _prog_cache[key]


def kernel(A, B, batch_sizes, batch_offsets, batch_padded_offsets):
    global LAST_EXEC_NS
    import ml_dtypes
    from concourse.bass_utils import run_bass_kernel_spmd

    try:
        import antenv.axon_hooks  # noqa: F401
    except ImportError:
        import sys
        import types

        _m = types.ModuleType("antenv.axon_hooks")
        _m.get_axon_ntff_profile_hook = lambda: None
        sys.modules.setdefault("antenv.axon_hooks", _m)

    kn = _knobs()
    bf16 = ml_dtypes.bfloat16
    A = np.asarray(A, dtype=np.float32)
    B = np.asarray(B, dtype=np.float32)
    sizes = np.asarray(batch_sizes, dtype=np.int64)
    offsets = np.asarray(batch_offsets, dtype=np.int64)

    M = A.shape[0]
    G = B.shape[0]
    bulk, clean = _schedule(sizes)
    r_list, plan = _order_slots(bulk, clean, kn["cleanat"])
    starts = np.concatenate([[0], np.cumsum(r_list)[:-1]]).astype(np.int64)
    T = int(sum(r_list))
    R = len(r_list)

    nc = _get_program(tuple(int(x) for x in r_list), kn)

    ATfull = np.ascontiguousarray(A.astype(bf16).T)  # [K, M]
    Bbf = B.astype(bf16)  # [G, K, N]

    in_maps = []
    for c in range(NCORES):
        at = np.zeros((K, T), dtype=bf16)
        bw = np.zeros((128, R, 2, 2, 128), dtype=bf16)
        for i in range(R):
            gid, gr0, nrows = plan[c][i]
            dst = int(starts[i])
            if gid < G:
                off, sz = int(offsets[gid]), int(sizes[gid])
                lo = min(gr0, sz)
                hi = min(gr0 + nrows, sz)
                if hi > lo:
                    at[:, dst + (lo - gr0) : dst + (hi - gr0)] = ATfull[
                        :, off + lo : off + hi
                    ]
                bw[:, i] = Bbf[gid].reshape(2, 128, 2, 128).transpose(1, 0, 2, 3)
        in_maps.append({"AT": at, "BW": bw})

    trace = bool(int(os.environ.get("BASS_GG_TRACE", "0")))
    repeats = int(os.environ.get("BASS_GG_REPEAT", "1"))
    times = []
    for _ in range(max(1, repeats)):
        res = run_bass_kernel_spmd(
            nc, in_maps, core_ids=list(range(NCORES)), trace=trace
        )
        times.append(res.exec_time_ns)
    global LAST_EXEC_LIST
    LAST_EXEC_LIST = times
    LAST_EXEC_NS = min((t for t in times if t is not None), default=None)

    outT = np.zeros((N, M), dtype=np.float32)
    for c in range(NCORES):
        oc = res.results[c]["OUTT"]
        for i in range(R):
            gid, gr0, nrows = plan[c][i]
            src = int(starts[i])
            if gid >= G:
                continue
            off, sz = int(offsets[gid]), int(sizes[gid])
            lo = min(gr0, sz)
            hi = min(gr0 + nrows, sz)
            if hi > lo:
                outT[:, off + lo : off + hi] = oc[
                    :, src + (lo - gr0) : src + (hi - gr0)
                ]
    return outT.T


# revision 11
# speedup vs baseline: 1.1260x; 1.0024x over previous
"""Grouped GEMM (MoE routing) kernel for 8 Trainium2 NeuronCores.

out[off_g : off_g + size_g] = A[off_g : off_g + size_g] @ B[g]   for g in 0..63
A: [524288, 256] f32, B: [64, 256, 256] f32, groups are contiguous row ranges.

Strategy (hardcoded, from the sharding hint "expert-parallel"):
  - Every core runs an IDENTICAL static schedule of T rows; the per-core
    group assignment lives in the DATA (host-packed AT / BW tensors).
  - Packing: constrained LPT assigns 8 groups per core (balanced totals),
    then a local-search swap pass minimizes T.  Slot structure = 8 "bulk"
    slots (budget = min of the i-th largest group over cores) + ~7
    "cleanup" slots (sorted leftover tails, budget = max over cores).
    T ~ 66240 rows (1.1% padding) vs 69632 for one-group-per-slot octiles.
  - bf16 operands + bf16 output (accumulation stays f32 in PSUM).
  - Host packs each core's chunks back to back, pre-transposed to
    AT [256, T] bf16; device computes the TRANSPOSED output OUTT [256, T].
  - Uniform 2K-row blocks tapered at both ends (fine blocks + 10-deep
    load / 8-deep store buffering ride out the chip's ~50%-util power
    throttle windows, which otherwise add 20-40us; weights after the
    first chunk are deferred past the fill window).  Per segment and output
    half h (128 of 256 N cols): stream <=512-row matmul spans (one PSUM
    bank each; h=0 rotates banks 0-3, h=1 banks 4-7), then cast-copy
    PSUM->SBUF on vector (h=0) / scalar (h=1) so both drain concurrently.
    GROUPN=2 spans per PSUM group: shorter PE bursts interleave better
    with the copy engines and, empirically, largely avoid the ~50%-util
    activity-throttle windows that add ~25us to GROUPN=4 runs.
  - DMA queue topology is selectable (BASS_GG_TOPO); the last TAILROT
    blocks rotate stores over all 3 queues (loads are done by then) to
    speed the drain.
"""

import os
import numpy as np

NCORES = 8
K = 256
N = 256
SPAN = 512       # rows per PSUM bank (512 f32 = 2KB = one bank)

LAST_EXEC_NS = None  # set when BASS_GG_TRACE=1
LAST_EXEC_LIST = []

_prog_cache = {}
_sched_cache = {}


def _knobs():
    e = os.environ.get
    return dict(
        wrows=int(e("BASS_GG_W", "2048")),
        groupn=int(e("BASS_GG_GROUP", "2")),
        abufs=int(e("BASS_GG_ABUFS", "10")),
        obufs=int(e("BASS_GG_OBUFS", "8")),
        lookahead=int(e("BASS_GG_LOOKAHEAD", "5")),
        tailrot=int(e("BASS_GG_TAILROT", "6")),
        lead=tuple(int(x) for x in e("BASS_GG_LEAD", "1024,1024").split(",") if x),
        tail=tuple(int(x) for x in e("BASS_GG_TAIL", "1024,512,512").split(",") if x),
        cleanat=float(e("BASS_GG_CLEANAT", "0.55")),
        topo=e("BASS_GG_TOPO", "mixed"),
        ssplit=int(e("BASS_GG_SSPLIT", "1")),
        wdefer=int(e("BASS_GG_WDEFER", "1")),
    )


def _schedule(sizes):
    """Pack groups into an identical-per-core slot schedule.

    Returns (r_list_raw, plan) where r_list_raw is the UNORDERED slot
    budget list [(budget, kind)] and plan[c][slot] = (group_id,
    group_row_start, nrows).  Groups padded to 64-row multiples.
    """
    import random

    key = tuple(int(x) for x in sizes)
    if key in _sched_cache:
        return _sched_cache[key]

    sizes = np.asarray(sizes, dtype=np.int64)
    g = sizes.shape[0]
    pad_groups = (-g) % NCORES
    if pad_groups:
        sizes = np.concatenate([sizes, np.zeros(pad_groups, np.int64)])
        g += pad_groups
    s64 = ((sizes + 63) // 64 * 64).astype(np.int64)
    per = g // NCORES

    order = np.argsort(-s64, kind="stable")
    tot = np.zeros(NCORES, np.int64)
    cnt = np.zeros(NCORES, np.int64)
    assign = [[] for _ in range(NCORES)]
    for gid in order:
        elig = [c for c in range(NCORES) if cnt[c] < per]
        c = min(elig, key=lambda c: tot[c])
        tot[c] += s64[gid]
        cnt[c] += 1
        assign[c].append(int(gid))

    def stats(asg):
        S = np.array(
            [sorted((s64[x] for x in asg[c]), reverse=True) for c in range(NCORES)],
            dtype=np.int64,
        )
        q = S.min(axis=0)
        L = S - q[None, :]
        p = (-np.sort(-L, axis=1)).max(axis=0)
        return int(q.sum() + p[p > 0].sum()), q, p

    best, _, _ = stats(assign)
    rnd = random.Random(7)
    cur = [list(a) for a in assign]
    for _ in range(12000):
        c1, c2 = rnd.sample(range(NCORES), 2)
        i1, i2 = rnd.randrange(per), rnd.randrange(per)
        cur[c1][i1], cur[c2][i2] = cur[c2][i2], cur[c1][i1]
        t, _, _ = stats(cur)
        if t <= best:
            best = t
        else:
            cur[c1][i1], cur[c2][i2] = cur[c2][i2], cur[c1][i1]

    T, q, p = stats(cur)
    nb = len(q)
    Gc = [sorted(cur[c], key=lambda x: -s64[x]) for c in range(NCORES)]
    keepp = [int(x) for x in p if x > 0]
    ncl = len(keepp)
    bulk = []
    for i in range(nb):
        entries = [(Gc[c][i], 0, int(q[i])) for c in range(NCORES)]
        bulk.append((int(q[i]), entries))
    lsorted = []
    for c in range(NCORES):
        lv = [(int(s64[Gc[c][i]] - q[i]), i) for i in range(nb)]
        lv.sort(key=lambda x: -x[0])
        lsorted.append(lv)
    clean = []
    for j in range(ncl):
        entries = []
        for c in range(NCORES):
            l, i = lsorted[c][j]
            entries.append((Gc[c][i], int(q[i]), l))
        clean.append((keepp[j], entries))

    _sched_cache[key] = (bulk, clean)
    return bulk, clean


def _order_slots(bulk, clean, cleanat):
    T = sum(b[0] for b in bulk) + sum(c[0] for c in clean)
    slots = []
    cum = 0
    inserted = False
    for bslot in bulk:
        if not inserted and cum >= cleanat * T:
            slots.extend(clean)
            inserted = True
        slots.append(bslot)
        cum += bslot[0]
    if not inserted:
        slots.extend(clean)
    r_list = [s[0] for s in slots]
    plan = [[s[1][c] for s in slots] for c in range(NCORES)]
    return r_list, plan


def _make_blocks(r_list, kn):
    T = int(sum(r_list))
    lead = list(kn["lead"])
    tail = list(kn["tail"])
    wrows = kn["wrows"]
    mid = T - sum(lead) - sum(tail)
    parts = max(1, (mid + wrows - 1) // wrows)
    base = (mid // parts + 63) // 64 * 64
    sizes = []
    rem = mid
    while rem > 0:
        w = min(base, rem)
        sizes.append(w)
        rem -= w
    blocks = []
    t0 = 0
    for w in lead + sizes + tail:
        blocks.append((t0, w))
        t0 += w
    assert t0 == T
    return blocks, T


def _build_program(r_list, kn):
    import concourse.tile as tile
    from concourse import bacc, mybir

    BF16 = mybir.dt.bfloat16
    F32 = mybir.dt.float32
    R = len(r_list)
    WROWS = kn["wrows"]
    GROUPN = kn["groupn"]
    TAILROT = kn["tailrot"]
    LOOKAHEAD = kn["lookahead"]

    blocks, T = _make_blocks(r_list, kn)
    slot_start = [0]
    for r in r_list:
        slot_start.append(slot_start[-1] + int(r))

    def slot_at(row):
        for i in range(R):
            if row < slot_start[i + 1]:
                return i
        return R - 1

    nc = bacc.Bacc(
        "TRN2",
        target_bir_lowering=False,
        debug=False,
        enable_asserts=False,
        num_devices=NCORES,
    )
    AT = nc.dram_tensor("AT", [K, T], BF16, kind="ExternalInput").ap()
    BW = nc.dram_tensor("BW", [128, R, 2, 2, 128], BF16, kind="ExternalInput").ap()
    OUTT = nc.dram_tensor("OUTT", [N, T], BF16, kind="ExternalOutput").ap()

    with tile.TileContext(nc) as tc:
        with tc.tile_pool(name="bpool", bufs=1) as bpool, \
             tc.tile_pool(name="apool", bufs=kn["abufs"]) as apool, \
             tc.tile_pool(name="opool", bufs=kn["obufs"]) as opool, \
             tc.tile_pool(name="psum", bufs=8, space="PSUM") as pspool:
            b_sb = bpool.tile([128, R, 2, 2, 128], BF16)

            abufs = {}
            ENG = [nc.sync, nc.scalar, nc.gpsimd]
            topo = kn["topo"]
            if topo == "split":
                qa0, qa1 = nc.gpsimd, nc.gpsimd
                qh0, qh1 = nc.sync, nc.sync
                wq = [nc.scalar, nc.scalar, nc.scalar]
            elif topo == "split2":
                qa0, qa1 = nc.gpsimd, nc.scalar
                qh0, qh1 = nc.sync, nc.sync
                wq = [nc.sync, nc.scalar, nc.gpsimd]
            else:  # mixed (baseline-like)
                qa0, qa1 = nc.sync, nc.scalar
                qh0, qh1 = nc.gpsimd, nc.sync
                wq = [nc.gpsimd, nc.scalar, nc.sync]

            def emit_loads(bi):
                t0, w = blocks[bi]
                a0 = apool.tile([128, WROWS], BF16, tag="a0")
                a1 = apool.tile([128, WROWS], BF16, tag="a1")
                qa0.dma_start(out=a0[:, :w], in_=AT[0:128, t0 : t0 + w])
                qa1.dma_start(out=a1[:, :w], in_=AT[128:256, t0 : t0 + w])
                abufs[bi] = (a0, a1)

            nblk = len(blocks)

            def emit_compute(bi):
                t0, w = blocks[bi]
                a0, a1 = abufs.pop(bi)
                ob = opool.tile([128, 2, WROWS], BF16, tag="ob")

                segs = []
                off = 0
                while off < w:
                    s = slot_at(t0 + off)
                    end = min(w, slot_start[s + 1] - t0)
                    segs.append((off, end, s))
                    off = end

                for (so, se, s) in segs:
                    spans = []
                    off = so
                    while off < se:
                        spans.append((off, min(SPAN, se - off)))
                        off += spans[-1][1]
                    for c0 in range(0, len(spans), GROUPN):
                        chunk = spans[c0 : c0 + GROUPN]
                        for h in range(2):
                            pss = [
                                pspool.tile([128, SPAN], F32, name="ps")
                                for _ in chunk
                            ]
                            for j, aj in ((0, a0), (1, a1)):
                                for (off, ln), ps in zip(chunk, pss):
                                    nc.tensor.matmul(
                                        ps[:, :ln],
                                        lhsT=b_sb[:, s, j, h, :],
                                        rhs=aj[:, off : off + ln],
                                        start=(j == 0),
                                        stop=(j == 1),
                                    )
                            eng = (
                                nc.vector.tensor_copy if h == 0 else nc.scalar.copy
                            )
                            for (off, ln), ps in zip(chunk, pss):
                                eng(out=ob[:, h, off : off + ln], in_=ps[:, :ln])
                intail = TAILROT and bi >= nblk - TAILROT
                if intail:
                    # rotate from the end so the final block always lands on
                    # the two HWDGE rings (sync+scalar: fastest trigger and
                    # completion), the 2nd-to-last on gpsimd+sync, ...
                    k = nblk - 1 - bi
                    qs0 = ENG[(2 * k) % 3]
                    qs1 = ENG[(2 * k + 1) % 3]
                else:
                    qs0, qs1 = qh0, qh1
                nsp = kn["ssplit"] if w >= 2048 else 1
                cuts = [w * i // nsp // 64 * 64 for i in range(nsp)] + [w]
                for ci in range(nsp):
                    lo, hi = cuts[ci], cuts[ci + 1]
                    for h, qq in ((0, qs0), (1, qs1)):
                        if intail:
                            qq = ENG[(2 * bi + 2 * ci + h) % 3]
                        qq.dma_start(
                            out=OUTT[h * 128 : (h + 1) * 128, t0 + lo : t0 + hi],
                            in_=ob[:, h, lo:hi],
                        )

            # head: first two blocks' loads before the weights; weights in
            # schedule order so early slots arrive first
            emit_loads(0)
            emit_loads(1)
            cuts = [0, min(2, R), min(6, R), min(11, R), R]
            wqs = [wq[0], wq[0], wq[1], wq[2]]
            wchunks = [
                ((lo, hi), qq)
                for (lo, hi), qq in zip(zip(cuts, cuts[1:]), wqs)
                if hi > lo
            ]
            if kn["wdefer"]:
                # only the first chunk (earliest slots) before block 2's
                # loads; defer the rest past the fill window (they are not
                # consumed until ~1/3 into the schedule)
                now, later = wchunks[:1], wchunks[1:]
            else:
                now, later = wchunks, []
            for (lo, hi), qq in now:
                qq.dma_start(out=b_sb[:, lo:hi], in_=BW[:, lo:hi])

            for bi in range(2, nblk + LOOKAHEAD):
                if bi < nblk:
                    emit_loads(bi)
                if later and bi >= 3:
                    (lo, hi), qq = later.pop(0)
                    qq.dma_start(out=b_sb[:, lo:hi], in_=BW[:, lo:hi])
                if bi >= LOOKAHEAD:
                    emit_compute(bi - LOOKAHEAD)
    nc.compile()
    return nc


def _get_program(r_key, kn):
    key = (r_key, tuple(sorted(kn.items())))
    if key not in _prog_cache:
        _prog_cache[key] = _build_program(list(r_key), kn)
    return _prog_cache[key]


def kernel(A, B, batch_sizes, batch_offsets, batch_padded_offsets):
    global LAST_EXEC_NS
    import ml_dtypes
    from concourse.bass_utils import run_bass_kernel_spmd

    try:
        import antenv.axon_hooks  # noqa: F401
    except ImportError:
        import sys
        import types

        _m = types.ModuleType("antenv.axon_hooks")
        _m.get_axon_ntff_profile_hook = lambda: None
        sys.modules.setdefault("antenv.axon_hooks", _m)

    kn = _knobs()
    bf16 = ml_dtypes.bfloat16
    A = np.asarray(A, dtype=np.float32)
    B = np.asarray(B, dtype=np.float32)
    sizes = np.asarray(batch_sizes, dtype=np.int64)
    offsets = np.asarray(batch_offsets, dtype=np.int64)

    M = A.shape[0]
    G = B.shape[0]
    bulk, clean = _schedule(sizes)
    r_list, plan = _order_slots(bulk, clean, kn["cleanat"])
    starts = np.concatenate([[0], np.cumsum(r_list)[:-1]]).astype(np.int64)
    T = int(sum(r_list))
    R = len(r_list)

    nc = _get_program(tuple(int(x) for x in r_list), kn)

    ATfull = np.ascontiguousarray(A.astype(bf16).T)  # [K, M]
    Bbf = B.astype(bf16)  # [G, K, N]

    in_maps = []
    for c in range(NCORES):
        at = np.zeros((K, T), dtype=bf16)
        bw = np.zeros((128, R, 2, 2, 128), dtype=bf16)
        for i in range(R):
            gid, gr0, nrows = plan[c][i]
            dst = int(starts[i])
            if gid < G:
                off, sz = int(offsets[gid]), int(sizes[gid])
                lo = min(gr0, sz)
                hi = min(gr0 + nrows, sz)
                if hi > lo:
                    at[:, dst + (lo - gr0) : dst + (hi - gr0)] = ATfull[
                        :, off + lo : off + hi
                    ]
                bw[:, i] = Bbf[gid].reshape(2, 128, 2, 128).transpose(1, 0, 2, 3)
        in_maps.append({"AT": at, "BW": bw})

    trace = bool(int(os.environ.get("BASS_GG_TRACE", "0")))
    repeats = int(os.environ.get("BASS_GG_REPEAT", "1"))
    times = []
    for _ in range(max(1, repeats)):
        res = run_bass_kernel_spmd(
            nc, in_maps, core_ids=list(range(NCORES)), trace=trace
        )
        times.append(res.exec_time_ns)
    global LAST_EXEC_LIST
    LAST_EXEC_LIST = times
    LAST_EXEC_NS = min((t for t in times if t is not None), default=None)

    outT = np.zeros((N, M), dtype=np.float32)
    for c in range(NCORES):
        oc = res.results[c]["OUTT"]
        for i in range(R):
            gid, gr0, nrows = plan[c][i]
            src = int(starts[i])
            if gid >= G:
                continue
            off, sz = int(offsets[gid]), int(sizes[gid])
            lo = min(gr0, sz)
            hi = min(gr0 + nrows, sz)
            if hi > lo:
                outT[:, off + lo : off + hi] = oc[
                    :, src + (lo - gr0) : src + (hi - gr0)
                ]
    return outT.T
